# revision 75
# baseline (speedup 1.0000x reference)
"""Multi-head causal attention on 8 TRN2 NeuronCores.

Problem: x[4,2048,1024] @ Wqkv.T -> 16-head causal attention -> @ Wout.T.

Sharding: core c handles batch b=c//2, head-group g=c%2 (8 heads of 64).
Each core computes qkv for its (batch, head-group) slice, causal attention,
and a partial out-projection over its 512 columns of Wout's input dim.
Host sums the two partials per batch (the all-reduce of the hint).

Per-core layouts (host pre-transposes so every matmul contraction dim lands
on SBUF partitions):
  xT   [1024 d, 2048 t] bf16    wqkT [1024 d, 1024 (q|k)e] bf16
  wvT  [1024 d,  512 e] bf16    woT  [ 512 e, 1024 f] fp32r
x/Wqkv/Wv and P (post-exp)/V are bf16 (halves input DMA; QKV matmuls stream
at the same 1 cyc/row); Q/K/S/out-proj stay fp32r.  Measured rel err ~3e-3
against the 2e-2 gate.

Schedule notes (hard-won, from perfetto traces):
- The PE clock ramps (half speed until ~3us of continuous busy), so every
  idle gap costs double.  The attention inner loop pipelines
    step s:  S(s) | exp(s-1) on ACT | affine_select(s-1) on gpsimd | AV(s-7)
  so the ACT exp (~1us/block vs ~0.85us PE work per block) never gates the
  PE, and each pair's S-only head steps overlap the previous pair's
  normalize drain (AV lag 7).
- Cross-engine waits are conservative (an instruction waits the producing
  engine's whole queue up to its own emission point).  Hence: each pair's
  normalize is EMITTED at step 4 of the next pair (flush_norm), loads are
  emitted only just before their consumers (emit_wave_loads /
  emit_late_loads / xs after endfill), and the epilogue emits chains'
  m=0..2 members before the final normalize.
- Every dma_start costs ~600ns of serial descriptor-gen on its queue; the
  softmax denominator gather gens on the (idle-at-pair-end) gpsimd queue,
  the head-B partition move is a DVE stream_shuffle (no DMA), epilogue
  z-store gens split across the sync and scalar queues.
- PE filler (next-chunk QKV, prev-chunk out-proj) is distributed per
  window/pair so tile-pool slot rotation never waits on a later-emitted
  reader: w1 <- op(0); w3 <- op(1), op(2) and chunk-3's pair-3 qk (its
  pairs 0-2 don't read kt[3]/qt[3]); out-proj(3) is the epilogue with 6
  PSUM accumulators borrowed from the idle st pool.
- Diagonal j-blocks only stream/exp query columns >= 128r (clamped to
  N>=256 for fp32r); affine_select zeroes the masked+stale region;
  gpsimd/ACT/DVE cannot write across partitions (matmul PSUM writes must
  start at partition 0; partition_broadcast only from/to an offset-0 row),
  which dictates the [V|ones] denominator trick and the normalize shape.
"""

import sys

sys.path.insert(0, "/opt/trn_rl_repo")

import numpy as np

B, T, D, H = 4, 2048, 1024, 16
E = 512  # per-core head width (8 heads x 64)
ND = 8  # d chunks of 128
NTC = 4  # t chunks of 512
SCALE = 0.125  # 1/sqrt(64)
Q0R = [0, 128, 256, 256]  # first live query col per diag sub-block r
LAG = 7  # AV trails S by LAG j-blocks: the pair's S-only head
         # steps overlap the previous pair's normalize drain

_NC_CACHE = {}


def build():
    if "nc" in _NC_CACHE:
        return _NC_CACHE["nc"]
    import concourse.bacc as bacc
    import concourse.mybir as mybir
    import concourse.tile as tile

    F32 = mybir.dt.float32
    F32R = mybir.dt.float32r
    BF16 = mybir.dt.bfloat16
    EXP = mybir.ActivationFunctionType.Exp

    nc = bacc.Bacc("TRN2", target_bir_lowering=False, debug=False, num_devices=8)
    xT = nc.declare_dram_parameter("xT", [D, T], BF16, isOutput=False)
    wqkT = nc.declare_dram_parameter("wqkT", [D, 2 * E], BF16, isOutput=False)
    wvT = nc.declare_dram_parameter("wvT", [D, E], BF16, isOutput=False)
    woT = nc.declare_dram_parameter("woT", [E, D], F32R, isOutput=False)
    z = nc.declare_dram_parameter("z", [T, D], F32, isOutput=True)
    dbg = {}
    if _NC_CACHE.get("debug"):
        for nm, shp in [
            ("dpt", [128, 512]), ("dya", [96, 512]), ("drca", [1, 512]),
            ("drba", [64, 512]), ("drbb", [64, 512]), ("dysb", [128, 512]),
            ("dytm", [64, 512]),
        ]:
            dbg[nm] = nc.declare_dram_parameter(nm, shp, F32, isOutput=True)

    with tile.TileContext(nc) as tc_:
        with (
            tc_.tile_pool(name="pw", bufs=1) as pw,
            tc_.tile_pool(name="pwo", bufs=4) as pwo,
            tc_.tile_pool(name="px", bufs=4) as px,
            tc_.tile_pool(name="pkt", bufs=4) as pkt,
            tc_.tile_pool(name="pqt", bufs=4) as pqt,
            tc_.tile_pool(name="pv", bufs=16) as pv,
            tc_.tile_pool(name="ppt", bufs=8) as ppt,
            tc_.tile_pool(name="pr", bufs=2) as pr,
            tc_.tile_pool(name="pysb", bufs=10) as pysb,
            tc_.tile_pool(name="pzsb", bufs=4) as pzsb,
            tc_.tile_pool(name="pst", bufs=2, space="PSUM") as pst,
            tc_.tile_pool(name="pyd", bufs=1, space="PSUM") as pyd,
            tc_.tile_pool(name="pfa", bufs=2, space="PSUM") as pfa,
        ):
            # ---- weights + first x chunks, striped across DMA queues and
            # ordered so the dc=0 QKV chain can start within ~5us: the
            # prologue is input-DMA-bound, so emission order is load order.
            def dma_striped(dst, src, nstripe):
                w = dst.shape[1]
                sw = w // nstripe
                for s_ in range(nstripe):
                    nc.sync.dma_start(
                        dst[:, s_ * sw : (s_ + 1) * sw],
                        src[:, s_ * sw : (s_ + 1) * sw],
                    )

            def emit_x_loads(tci):
                # one descriptor-gen for the whole chunk (the sync
                # sequencer costs ~600ns per gen, serially)
                t0 = tci * 512
                xa = px.tile([128, ND * 512], BF16, tag="x", name="xs")
                src = xT[:, t0 : t0 + 512].rearrange(
                    "(dc p) t -> p dc t", p=128
                )
                dst = xa[:].rearrange("p (dc t) -> p dc t", dc=ND)
                # two gens by dc-halves: consumers need the full t-range of
                # one dc, so half the chunks are usable after the first
                # transfer instead of waiting one monolithic single-queue DMA
                nc.sync.dma_start(dst[:, 0:4, :], src[:, 0:4, :])
                nc.sync.dma_start(dst[:, 4:8, :], src[:, 4:8, :])
                return [xa[:, dc * 512 : (dc + 1) * 512] for dc in range(ND)]

            # load order = consumption order: q0/q1 weight columns + x
            # first (the q0 chain starts ~4us in), then k0/k1 columns, wv
            # (v chains), the rest of wqk, then the next chunk's x and wo.
            wqka = pw.tile([128, ND * 2 * E], BF16, tag="wqk", name="wqka")
            wqk = [
                wqka[:, dc * 2 * E : (dc + 1) * 2 * E] for dc in range(ND)
            ]
            xs_pa = px.tile([128, ND * 512], BF16, tag="x", name="xs")
            xs_p = [xs_pa[:, dc * 512 : (dc + 1) * 512] for dc in range(ND)]
            def wqk_cols_load(c0, c1):
                # all dc rows of wqkT cols [c0,c1) in one descriptor-gen
                src = wqkT[:, c0:c1].rearrange("(dc p) e -> p dc e", p=128)
                dst = wqka[:].rearrange("p (dc e) -> p dc e", dc=ND)[
                    :, :, c0:c1
                ]
                nc.sync.dma_start(dst, src)

            def emit_wave_loads(qk_chain_fn):
                # 4 gens bring in everything the q0/k0 chains need; the
                # chains follow immediately (their conservative sync
                # barrier covers only these gens)
                wqk_cols_load(0, 256)
                src = xT[:, 0:512].rearrange("(dc p) t -> p dc t", p=128)
                dst = xs_pa[:].rearrange("p (dc t) -> p dc t", dc=ND)
                nc.sync.dma_start(dst[:, :, 0:256], src[:, :, 0:256])
                nc.sync.dma_start(dst[:, :, 256:512], src[:, :, 256:512])
                wqk_cols_load(512, 768)
                for op in qk_chain_fn(0):
                    op()
                for op in qk_chain_fn(4):
                    op()

            wva = pw.tile([128, ND * E], BF16, tag="wv", name="wva")
            wv = [wva[:, dc * E : (dc + 1) * E] for dc in range(ND)]
            xs_by_tc = {0: xs_p}
            wo = [
                pwo.tile([128, D], F32R, tag="wo", name=f"wo{i}")
                for i in range(4)
            ]

            def emit_late_loads():
                # emitted only after the prologue chains: a consumer of any
                # DMA-produced tile waits on the whole sync queue up to its
                # emission point, so loads the prologue doesn't need must
                # not be queued ahead of it
                wqk_cols_load(256, 512)
                wqk_cols_load(768, 1024)
                xs_by_tc[1] = emit_x_loads(1)
                for m in range(4):
                    nc.sync.dma_start(
                        wo[m][:], woT[m * 128 : (m + 1) * 128, :]
                    )

            # persistent K^T [e,t] tiles; pair m = heads 2m / 2m+1 at
            # partition rows 0:64 / 64:128
            kt = [
                pkt.tile([128, T], F32R, tag="kt", name=f"kt{i}")
                for i in range(4)
            ]
            vt = [None] * 16  # bf16 [V_h(64)|ones(32)] per head, per j-block
            qt_by_tc = {}  # (tc, m) -> qt tile
            ysb_hist = {}  # (tc, m) -> normalized y^T tile

            # ---- filler chains (lists of zero-arg closures, one PE/DVE op
            # each, executed by the window scheduler's filler cursor)

            def qk_chain(xs, g, tci):
                """g 0..3: Q chunk for pair g; 4..7: K chunk for pair g-4."""
                ops = []
                state = {}

                def mk_mm(dc):
                    def run():
                        if "acc" not in state:
                            state["acc"] = pfa.tile(
                                [128, 512], F32, tag="facc", name="qkacc"
                            )
                        nc.tensor.matmul(
                            state["acc"][:],
                            wqk[dc][:, g * 128 : (g + 1) * 128],
                            xs[dc][:],
                            start=(dc == 0),
                            stop=(dc == ND - 1),
                        )

                    return run

                ops += [mk_mm(dc) for dc in range(ND)]

                def fin():
                    acc = state["acc"]
                    if g < 4:
                        t_ = pqt.tile([128, 512], F32R, tag="qt", name="qt")
                        nc.vector.tensor_copy(t_[:], acc[:])
                        qt_by_tc[(tci, g)] = t_
                    else:
                        t0 = tci * 512
                        nc.vector.tensor_copy(
                            kt[g - 4][:, t0 : t0 + 512], acc[:]
                        )

                ops.append(fin)
                return ops

            def v_chain(xs, tci, ts):
                jb = 4 * tci + ts
                ops = []
                state = {}

                def mk_mm(dc):
                    def run():
                        if "acc" not in state:
                            state["acc"] = pfa.tile(
                                [128, 512], F32, tag="facc", name="vacc"
                            )
                        nc.tensor.matmul(
                            state["acc"][:],
                            xs[dc][:, ts * 128 : (ts + 1) * 128],
                            wv[dc][:],
                            start=(dc == 0),
                            stop=(dc == ND - 1),
                        )

                    return run

                ops += [mk_mm(dc) for dc in range(ND)]

                def fin():
                    acc = state["acc"]
                    t_ = pv.tile([128, 768], BF16, tag="v", name="vt")
                    t4 = t_[:].rearrange("p (hh c) -> p hh c", hh=8)
                    a4 = acc[:].rearrange("p (hh c) -> p hh c", hh=8)
                    nc.vector.tensor_copy(t4[:, :, 0:64], a4[:])
                    nc.vector.memset(t4[:, :, 64:96], 1.0)
                    vt[jb] = t_

                ops.append(fin)
                return ops

            def outproj_chain(tcp, ib, fh, zp_tile=None, zp_cols=None):
                ops = []
                state = {}

                def get_zp():
                    if "zp" not in state:
                        if zp_tile is None:
                            state["zp"] = pfa.tile(
                                [128, 512], F32, tag="facc", name="zp"
                            )[:]
                        else:
                            state["zp"] = zp_tile[:][
                                :, zp_cols[0] : zp_cols[1]
                            ]
                    return state["zp"]

                def mk_mm(m):
                    def run():
                        nc.tensor.matmul(
                            get_zp(),
                            ysb_hist[(tcp, m)][:, ib * 128 : (ib + 1) * 128],
                            wo[m][:, fh * 512 : fh * 512 + 512],
                            start=(m == 0),
                            stop=(m == 3),
                        )

                    return run

                ops += [mk_mm(m) for m in range(4)]

                def fin():
                    zsb = pzsb.tile([128, 512], F32, tag="zsb")
                    nc.vector.tensor_copy(zsb[:], state["zp"])
                    row = (4 * tcp + ib) * 128
                    for s_ in range(2):
                        # epilogue stores split their descriptor-gens over
                        # the sync AND the (idle) scalar queue: gen is
                        # ~600ns serial per queue and paces the final drain
                        eng = (
                            nc.scalar
                            if (tcp == NTC - 1 and s_ == 1)
                            else nc.sync
                        )
                        eng.dma_start(
                            z[
                                row : row + 128,
                                fh * 512 + s_ * 256 : fh * 512 + (s_ + 1) * 256,
                            ],
                            zsb[:, s_ * 256 : (s_ + 1) * 256],
                        )

                ops.append(fin)
                return ops

            def outproj_chains(tcp):
                return [
                    outproj_chain(tcp, ib, fh)
                    for ib in range(4)
                    for fh in range(2)
                ]

            # ---- attention pieces
            def emit_S(tci, m, jb, qtm):
                st = pst.tile([128, 1024], F32, tag="st", name="st")
                r = jb - 4 * tci
                q0 = Q0R[r] if r >= 0 else 0
                for h in range(2):
                    nc.tensor.matmul(
                        st[:, h * 512 + q0 : (h + 1) * 512],
                        kt[m][
                            h * 64 : h * 64 + 64, jb * 128 : (jb + 1) * 128
                        ],
                        qtm[h * 64 : h * 64 + 64, q0:512],
                        start=True,
                        stop=True,
                    )
                return st

            def emit_exp(tci, jb, st):
                pt = ppt.tile([128, 1024], BF16, tag="pt", name="pt")
                r = jb - 4 * tci
                q0 = Q0R[r] if r >= 0 else 0
                if q0 > 0:
                    s3 = st[:].rearrange("p (h q) -> p h q", h=2)
                    p3 = pt[:].rearrange("p (h q) -> p h q", h=2)
                    nc.scalar.activation(
                        p3[:, :, q0:512], s3[:, :, q0:512], EXP, scale=SCALE
                    )
                    # cols < q0 are entirely masked and never exp'd
                    nc.vector.memset(p3[:, :, 0:q0], 0.0)
                else:
                    nc.scalar.activation(pt[:], st[:], EXP, scale=SCALE)
                if r >= 0:
                    # zero masked region in the live cols: keep where
                    # (q' + q0) - k - 128 r >= 0
                    p3 = pt[:].rearrange("p (h q) -> p h q", h=2)
                    nc.gpsimd.affine_select(
                        out=p3[:, :, q0:512],
                        in_=p3[:, :, q0:512],
                        compare_op=mybir.AluOpType.is_ge,
                        fill=0.0,
                        base=q0 - 128 * r,
                        pattern=[[0, 2], [1, 512 - q0]],
                        channel_multiplier=-1,
                    )
                return pt

            def emit_AV(m, jb, pt, ya, yb, first, last):
                nc.tensor.matmul(
                    ya[:],
                    vt[jb][:, m * 192 : m * 192 + 96],
                    pt[:, 0:512],
                    start=first,
                    stop=last,
                )
                nc.tensor.matmul(
                    yb[:],
                    vt[jb][:, m * 192 + 96 : m * 192 + 192],
                    pt[:, 512:1024],
                    start=first,
                    stop=last,
                )

            def emit_normalize(tci, m, ya, yb):
                if dbg and tci == 0 and m == 0:
                    dya_sb = pzsb.tile([128, 512], F32, tag="zsb")
                    nc.vector.tensor_copy(dya_sb[0:96, :], ya[:])
                    nc.sync.dma_start(dbg["dya"][:], dya_sb[0:96, :])
                # ya/yb rows 64:96 hold the softmax denominator (ones
                # columns of vt); reciprocal runs in-place at partition 64,
                # gpsimd broadcasts it down to rows 0:64.
                # critical path after the last AV: copies (DVE+gpsimd in
                # parallel) -> rc0 gather (gen on the idle gpsimd queue so
                # it can't head-of-line-block the sync DMA queue) -> recip
                # -> one wide broadcast -> muls (DVE+gpsimd) -> shuffle
                # (DVE partition move; no DMA anywhere near the hot path)
                rca = pr.tile([128, 1024], F32, tag="rca", bufs=1)
                nc.vector.tensor_copy(rca[64:65, 0:512], ya[64:65, :])
                nc.vector.tensor_copy(rca[64:65, 512:1024], yb[64:65, :])
                # partition-move head B's raw y to rows 64:128 (off the
                # critical path; the mul below does the fp32r rounding)
                ytmp = pr.tile([128, 512], F32, tag="ytmp", bufs=1)
                nc.vector.stream_shuffle(
                    ytmp[64:128, :], yb[0:64, :], mask=list(range(32))
                )
                rc0 = pr.tile([1, 1024], F32, tag="rc0", bufs=1)
                nc.gpsimd.dma_start(rc0[0:1, :], rca[64:65, :])
                rcv = pr.tile([1, 1024], F32, tag="rcv", bufs=1)
                nc.vector.reciprocal_approx_fast(rcv[0:1, :], rc0[0:1, :])
                rb = pr.tile([128, 1024], F32, tag="rb", bufs=1)
                nc.gpsimd.partition_broadcast(rb[:, :], rcv[0:1, :])
                ysb = pysb.tile([128, 512], F32R, tag="ysb", name="ysb")
                nc.vector.tensor_mul(
                    ysb[0:64, :], ya[0:64, :], rb[0:64, 0:512]
                )
                nc.vector.tensor_mul(
                    ysb[64:128, :], ytmp[64:128, :], rb[64:128, 512:1024]
                )
                if dbg and tci == 0 and m == 0:
                    nc.sync.dma_start(dbg["drca"][:], rcv[0:1, 0:512])
                    nc.sync.dma_start(dbg["drba"][:], rb[0:64, 0:512])
                    nc.sync.dma_start(dbg["drbb"][:], rb[64:128, 512:1024])
                    nc.sync.dma_start(dbg["dytm"][:], ytmp[64:128, :])
                    nc.sync.dma_start(dbg["dysb"][:], ysb[:].bitcast(F32))
                ysb_hist[(tci, m)] = ysb

            # ---- prologue: chunk 0 pair-0 q/k + all V, dense; the rest of
            # chunk 0's q/k production fills window 0's pairs (the prologue
            # is input-DMA-bound, so attention starts as soon as pair 0's
            # inputs exist)
            xs_cur = xs_by_tc[0]
            emit_wave_loads(lambda g: qk_chain(xs_cur, g, 0))
            nc.sync.dma_start(
                wva[:].rearrange("p (dc e) -> p dc e", dc=ND),
                wvT[:].rearrange("(dc p) e -> p dc e", p=128),
            )
            for ts in range(4):
                for op in v_chain(xs_cur, 0, ts):
                    op()
            emit_late_loads()

            # ---- windows
            pending_norm = [None]

            def flush_norm():
                if pending_norm[0] is not None:
                    pending_norm[0]()
                    pending_norm[0] = None

            for tci in range(NTC):
                njb = 4 * tci + 4
                pairfill = [[], [], [], []]
                endfill = []
                if tci == 0:
                    # rest of chunk 0's q/k: pair p produces pair p+1's
                    for p_ in range(3):
                        pairfill[p_] += qk_chain(xs_cur, p_ + 1, 0)
                        pairfill[p_] += qk_chain(xs_cur, p_ + 5, 0)
                if tci + 1 < NTC:
                    xs_next = xs_by_tc[tci + 1]
                    if tci < 2:
                        for g in range(3):
                            pairfill[g] += qk_chain(xs_next, g, tci + 1)
                            pairfill[g] += qk_chain(xs_next, g + 4, tci + 1)
                            pairfill[g + 1] += v_chain(xs_next, tci + 1, g)
                        endfill += qk_chain(xs_next, 3, tci + 1)
                        endfill += qk_chain(xs_next, 7, tci + 1)
                        endfill += v_chain(xs_next, tci + 1, 3)
                        if tci == 1:
                            # after the qk chains so the early pair-0 steps
                            # carry no ysb-dependent work
                            for i, ch in enumerate(outproj_chains(0)):
                                pairfill[1 + i % 3] += ch
                    else:
                        # window 2: chunk 3's qkv minus its pair-3 qk, which
                        # runs inside window 3 (whose pairs 0-2 don't touch
                        # kt[3]/qt[3]); out-proj(1) and (2) also defer to
                        # window 3, whose ACT-paced steps need PE filler
                        pairfill[0] += v_chain(xs_next, tci + 1, 0)
                        pairfill[0] += v_chain(xs_next, tci + 1, 1)
                        for g in range(3):
                            pairfill[g + 1] += qk_chain(xs_next, g, tci + 1)
                            pairfill[g + 1] += qk_chain(
                                xs_next, g + 4, tci + 1
                            )
                        pairfill[3] += v_chain(xs_next, tci + 1, 2)
                        endfill += v_chain(xs_next, tci + 1, 3)
                        _w3qk = qk_chain(xs_next, 3, tci + 1) + qk_chain(
                            xs_next, 7, tci + 1
                        )
                else:
                    op1 = outproj_chains(1)
                    op2 = outproj_chains(2)
                    pairfill[0] = _w3qk + op1[0] + op1[1]
                    pairfill[1] = op1[2] + op1[3] + op1[4] + op1[5]
                    pairfill[2] = op1[6] + op1[7] + op2[0] + op2[1] + op2[2]
                    pairfill[3] = op2[3] + op2[4] + op2[5] + op2[6] + op2[7]

                for m in range(4):
                    qtm = qt_by_tc[(tci, m)]
                    ya = yb = None
                    fl = pairfill[m]
                    fcur = 0
                    nsteps = njb + LAG
                    ring_st = [None] * 2
                    ring_pt = [None] * 8
                    for s in range(nsteps):
                        if s < njb:
                            ring_st[s % 2] = emit_S(tci, m, s, qtm)
                        if 1 <= s <= njb:
                            jb = s - 1
                            ring_pt[jb % 8] = emit_exp(
                                tci, jb, ring_st[jb % 2]
                            )
                            if dbg and tci == 0 and m == 0 and jb == 0:
                                nc.sync.dma_start(
                                    dbg["dpt"][:],
                                    ring_pt[0][:].bitcast(F32),
                                )
                        if s == 4:
                            # the prev pair's normalize is emitted only now:
                            # this pair's S/exp head sits BEFORE it in every
                            # engine queue, so the conservative cross-engine
                            # sems don't stall the PE through the drain
                            flush_norm()
                        if s == 5:
                            ya = pyd.tile([96, 512], F32, tag="ya", bufs=1)
                            yb = pyd.tile([96, 512], F32, tag="yb", bufs=1)
                        if LAG <= s < njb + LAG:
                            jb = s - LAG
                            emit_AV(
                                m,
                                jb,
                                ring_pt[jb % 8],
                                ya,
                                yb,
                                jb == 0,
                                jb == njb - 1,
                            )
                        target = len(fl) * (s + 1) // nsteps
                        while fcur < target:
                            fl[fcur]()
                            fcur += 1
                    pending_norm[0] = (
                        lambda tci=tci, m=m, ya=ya, yb=yb: emit_normalize(
                            tci, m, ya, yb
                        )
                    )
                for op in endfill:
                    op()
                # after the endfill: its matmuls' sync-queue barrier must
                # not cover these transfers
                if tci + 2 < NTC:
                    xs_by_tc[tci + 2] = emit_x_loads(tci + 2)

            # ---- epilogue: last chunk's out-projection. The st-pool banks
            # are free now, so 6 zp accumulators keep 6 chains in flight
            # (the 2-slot facc rotation alone would half-idle the PE here).
            ezp = [
                pst.tile([128, 1024], F32, tag="st", name=f"ezp{i}")
                for i in range(2)
            ]
            chains = []
            k_ = 0
            for ib in range(4):
                for fh in range(2):
                    if 2 <= k_ <= 5:
                        tile_ = ezp[(k_ - 2) // 2]
                        cols = ((k_ % 2) * 512, (k_ % 2) * 512 + 512)
                        chains.append(outproj_chain(NTC - 1, ib, fh, tile_, cols))
                    else:
                        chains.append(outproj_chain(NTC - 1, ib, fh))
                    k_ += 1
            # chains 0-5's m=0..2 members read only ysb(3,0..2), so they
            # run during the final pair's normalize drain; the m=3 members
            # and chains 6-7 follow the flush
            for ch in chains[0:6]:
                for op in ch[0:3]:
                    op()
            flush_norm()
            for ch in chains[0:6]:
                for op in ch[3:]:
                    op()
            for ch in chains[6:8]:
                for op in ch:
                    op()

    nc.finalize()
    _NC_CACHE["nc"] = nc
    return nc


def _in_maps(x, Wqkv, Wout):
    import ml_dtypes

    bf16 = ml_dtypes.bfloat16
    x = np.ascontiguousarray(np.asarray(x, dtype=np.float32))
    Wqkv = np.ascontiguousarray(np.asarray(Wqkv, dtype=np.float32))
    Wout = np.ascontiguousarray(np.asarray(Wout, dtype=np.float32))
    xTs = [np.ascontiguousarray(x[b].T.astype(bf16)) for b in range(B)]
    maps = []
    for c in range(8):
        b, g = divmod(c, 2)
        qrows = Wqkv[E * g : E * g + E]
        krows = Wqkv[D + E * g : D + E * g + E]
        vrows = Wqkv[2 * D + E * g : 2 * D + E * g + E]
        maps.append(
            {
                "xT": xTs[b],
                "wqkT": np.ascontiguousarray(
                    np.concatenate([qrows, krows], axis=0).T.astype(bf16)
                ),
                "wvT": np.ascontiguousarray(vrows.T.astype(bf16)),
                "woT": np.ascontiguousarray(Wout[:, E * g : E * g + E].T),
            }
        )
    return maps


def _run(x, Wqkv, Wout, trace=False):
    from concourse.bass_utils import run_bass_kernel_spmd

    nc = build()
    res = run_bass_kernel_spmd(
        nc, _in_maps(x, Wqkv, Wout), core_ids=list(range(8)), trace=trace
    )
    out = np.empty((B, T, D), dtype=np.float32)
    for b in range(B):
        out[b] = res.results[2 * b]["z"] + res.results[2 * b + 1]["z"]
    return out, res


def kernel(x, Wqkv, Wout):
    out, _ = _run(x, Wqkv, Wout, trace=False)
    return out
